# revision 6
# baseline (speedup 1.0000x reference)
"""Trainium2 Bass kernel for the leaky-tanh recurrent network (CombinedModel).

Math (reference):
    u_seq = u.transpose(0,2,1).reshape(Iq*T, C)      # state persists across inquiries
    x_t = (1-dt)*x_{t-1} + (dt*W) @ tanh(x_{t-1}) + (dt*M) @ u_t
    y_t = sigmoid(H @ x_t + b)

Strategy: the recurrence is sequential, but solvable in blocks of B=256
steps with Picard iteration: given a guess trajectory X, the refined
trajectory is the affine scan x_t = a*x_{t-1} + (G_t + V_t) with
G = (dt*W) @ tanh(X_shifted) and V = (dt*M) @ U. The scan is one DVE/Pool
tensor_tensor_scan instruction per 128-channel half; G and V are matmuls.

Two levels of pipelining squeeze the serial chain:
  * fp16 matmul inputs (weights, u, tanh(x), readout) -- 1 PE cycle/row
    instead of 4 for fp32. Scans and state stay fp32, PSUM accumulates
    fp32, so only per-use rounding noise enters (validated ~2e-3 max err
    vs the 2e-2 tolerance in a bit-accurate numpy mimic).
  * cross-block overlap: block b+1 runs its first K-JL Picard iterations
    anchored on a SNAPSHOT of block b's state taken after iteration K-JL
    (already ~1e-4 accurate), then its last JL iterations re-anchor on
    block b's final state. Only JL=2 iterations per block are truly
    serial; the other 9 overlap with neighboring blocks, so the kernel
    is engine-throughput-bound instead of latency-bound.

Engine split per iteration: ACT does one merged tanh over both channel
halves; PE does I@V drive-injection plus the 4 half-matmuls of W@tanh;
the two half-scans run concurrently on DVE (h0) and Pool/GpSimd (h1).
"""

import numpy as np

INQ, C, T = 64, 256, 1024
B = 256            # picard block length
P = 128            # partitions; C = 2*P channel halves
K = 11             # picard refinement iterations per block
JL = 2             # late iterations re-anchored on the exact initial state
NBLK = INQ * T // B
BPI = T // B       # blocks per inquiry

_cache: dict = {}


def _build(dt: float, n_blocks: int, kit: int, jl: int):
    import concourse.bacc as bacc
    import concourse.mybir as mybir
    from concourse.tile import TileContext

    f32 = mybir.dt.float32
    f16 = mybir.dt.float16
    AF = mybir.ActivationFunctionType
    ALU = mybir.AluOpType
    a = 1.0 - dt

    nc = bacc.Bacc(None, target_bir_lowering=False, debug=False)
    u_in = nc.dram_tensor("u16", [NBLK, P, 2, B], f16, kind="ExternalInput")
    wt_in = nc.dram_tensor("wt16", [P, 2, C], f16, kind="ExternalInput")
    mt_in = nc.dram_tensor("mt16", [P, 2, C], f16, kind="ExternalInput")
    ht_in = nc.dram_tensor("ht16", [P, 2, C], f16, kind="ExternalInput")
    id_in = nc.dram_tensor("ident", [P, P], f16, kind="ExternalInput")
    b_in = nc.dram_tensor("bvec", [P, 2], f32, kind="ExternalInput")
    ys_out = nc.dram_tensor("ys16", [NBLK, P, 2, B], f16, kind="ExternalOutput")
    xs_out = nc.dram_tensor("xs16", [NBLK, P, 2, B], f16, kind="ExternalOutput")

    with TileContext(nc) as tc:
        with (
            tc.tile_pool(name="const", bufs=1) as cpool,
            tc.tile_pool(name="io", bufs=4) as iopool,
            tc.tile_pool(name="xp", bufs=3) as xpool,
            tc.tile_pool(name="txp", bufs=3) as txpool,
            tc.tile_pool(name="vp", bufs=3) as vpool,
            tc.tile_pool(name="x16p", bufs=2) as x16pool,
            tc.tile_pool(name="yp", bufs=2) as ypool,
            tc.tile_pool(name="snap", bufs=2) as snappool,
            tc.tile_pool(name="pc", bufs=3, space="PSUM") as pcpool,
            tc.tile_pool(name="pv", bufs=2, space="PSUM") as pvpool,
            tc.tile_pool(name="py", bufs=2, space="PSUM") as pypool,
        ):
            w_sb = cpool.tile([P, 2, C], f16, tag="w")
            m_sb = cpool.tile([P, 2, C], f16, tag="m")
            h_sb = cpool.tile([P, 2, C], f16, tag="h")
            i_sb = cpool.tile([P, P], f16, tag="i")
            b_sb = cpool.tile([P, 2], f32, tag="b")
            a_sb = cpool.tile([P, B], f32, tag="a")
            zsnap = cpool.tile([P, 2], f32, tag="z")
            nc.sync.dma_start(w_sb[:, :, :], wt_in[:, :, :])
            nc.sync.dma_start(m_sb[:, :, :], mt_in[:, :, :])
            nc.sync.dma_start(h_sb[:, :, :], ht_in[:, :, :])
            nc.sync.dma_start(i_sb[:, :], id_in[:, :])
            nc.sync.dma_start(b_sb[:, :], b_in[:, :])
            nc.vector.memset(a_sb[:, :], a)
            nc.vector.memset(zsnap[:, :], 0.0)

            def emit_load(blk):
                u_t = iopool.tile([P, 2, B], f16, tag="u")
                nc.sync.dma_start(u_t[:, :, :], u_in[blk])
                return u_t

            def emit_drive(u_t):
                # V = (dt*M) @ U into psum; fp16 SBUF copy for the I@V injection
                pv = pvpool.tile([P, 2, B], f32, tag="pv")
                for mh in range(2):
                    for kh in range(2):
                        nc.tensor.matmul(
                            pv[:, mh, :],
                            m_sb[:, kh, mh * P : (mh + 1) * P],
                            u_t[:, kh, :],
                            start=(kh == 0),
                            stop=(kh == 1),
                        )
                v16 = vpool.tile([P, 2, B], f16, tag="v")
                nc.gpsimd.tensor_scalar_add(v16[:, :, :], pv[:, :, :], 0.0)
                return pv, v16

            def emit_scan0(pv, anchor):
                # col0 <- stale anchor; x^0 = affine scan of the drive alone
                x_t = xpool.tile([P, 2, B + 1], f32, tag="x")
                nc.gpsimd.tensor_scalar_add(x_t[:, :, 0:1], anchor[:, :, None], 0.0)
                nc.vector.tensor_tensor_scan(
                    x_t[:, 0, 1 : B + 1], a_sb[:, :], pv[:, 0, :],
                    x_t[:, 0, 0:1], ALU.mult, ALU.add,
                )
                nc.gpsimd.tensor_tensor_scan(
                    x_t[:, 1, 1 : B + 1], a_sb[:, :], pv[:, 1, :],
                    x_t[:, 1, 0:1], ALU.mult, ALU.add,
                )
                return x_t

            def emit_iter(x_t, v16):
                pcb = pcpool.tile([P, 2, B], f32, tag="pc")
                pc0, pc1 = pcb[:, 0, :], pcb[:, 1, :]
                nc.tensor.matmul(pc0[:, :], i_sb[:, :], v16[:, 0, :], start=True, stop=False)
                nc.tensor.matmul(pc1[:, :], i_sb[:, :], v16[:, 1, :], start=True, stop=False)
                tx = txpool.tile([P, 2, B], f16, tag="tx")
                nc.scalar.activation(tx[:, :, :], x_t[:, :, 0:B], AF.Tanh)
                pc = (pc0, pc1)
                for kh in range(2):
                    for mh in range(2):
                        nc.tensor.matmul(
                            pc[mh][:, :],
                            w_sb[:, kh, mh * P : (mh + 1) * P],
                            tx[:, kh, :],
                            start=False,
                            stop=(kh == 1),
                        )
                nc.vector.tensor_tensor_scan(
                    x_t[:, 0, 1 : B + 1], a_sb[:, :], pc0[:, :],
                    x_t[:, 0, 0:1], ALU.mult, ALU.add,
                )
                nc.gpsimd.tensor_tensor_scan(
                    x_t[:, 1, 1 : B + 1], a_sb[:, :], pc1[:, :],
                    x_t[:, 1, 0:1], ALU.mult, ALU.add,
                )

            def emit_snapshot(x_t):
                xsnap = snappool.tile([P, 2], f32, tag="s")
                nc.gpsimd.tensor_scalar_add(xsnap[:, :], x_t[:, :, B], 0.0)
                return xsnap

            def emit_switch(x_t, x_prev):
                # re-anchor col0 on the previous block's final state
                if x_prev is None:
                    nc.gpsimd.tensor_scalar_add(x_t[:, :, 0:1], zsnap[:, :, None], 0.0)
                else:
                    nc.gpsimd.tensor_scalar_add(x_t[:, :, 0:1], x_prev[:, :, B, None], 0.0)

            def emit_readout(blk, x_t):
                x16 = x16pool.tile([P, 2, B], f16, tag="x16")
                nc.vector.tensor_scalar_add(x16[:, :, :], x_t[:, :, 1 : B + 1], 0.0)
                py = pypool.tile([P, 2 * B], f32, tag="py")
                for mh in range(2):
                    for kh in range(2):
                        nc.tensor.matmul(
                            py[:, mh * B : (mh + 1) * B],
                            h_sb[:, kh, mh * P : (mh + 1) * P],
                            x16[:, kh, :],
                            start=(kh == 0),
                            stop=(kh == 1),
                        )
                y16 = ypool.tile([P, 2, B], f16, tag="y")
                for mh in range(2):
                    nc.scalar.activation(
                        y16[:, mh, :], py[:, mh * B : (mh + 1) * B],
                        AF.Sigmoid, bias=b_sb[:, mh : mh + 1],
                    )
                nc.sync.dma_start(ys_out[blk], y16[:, :, :])
                nc.sync.dma_start(xs_out[blk], x16[:, :, :])

            # ---- software-pipelined emission over blocks ---------------
            # round b emits: stale phase of block b (K-JL iters + snapshot),
            # then the late phase of block b-1 (re-anchor + JL iters +
            # readout), then drive+scan0 of block b+1.
            u_next = [emit_load(0), emit_load(1)]
            pv_cur, v_cur = emit_drive(u_next[0])
            x_cur = emit_scan0(pv_cur, zsnap)
            v_of = {0: v_cur}
            x_of = {0: x_cur}
            x_prev_final = None
            for b in range(n_blocks):
                if b + 2 < n_blocks:
                    u_next.append(emit_load(b + 2))
                x_t, v16 = x_of[b], v_of[b]
                for k in range(kit - jl):
                    emit_iter(x_t, v16)
                xsnap = emit_snapshot(x_t)
                if b > 0:
                    xp = x_of[b - 1]
                    emit_switch(xp, x_of.get(b - 2))
                    for k in range(jl):
                        emit_iter(xp, v_of[b - 1])
                    emit_readout(b - 1, xp)
                    x_of.pop(b - 2, None)
                    v_of.pop(b - 1, None)
                if b + 1 < n_blocks:
                    pv_n, v_n = emit_drive(u_next[b + 1])
                    v_of[b + 1] = v_n
                    x_of[b + 1] = emit_scan0(pv_n, xsnap)
            # epilogue: late phase of the last block
            bl = n_blocks - 1
            xp = x_of[bl]
            emit_switch(xp, x_of.get(bl - 1))
            for k in range(jl):
                emit_iter(xp, v_of[bl])
            emit_readout(bl, xp)

    nc.compile()
    return nc


def _get_nc(dt: float, n_blocks: int, kit: int, jl: int):
    key = (dt, n_blocks, kit, jl)
    if key not in _cache:
        _cache[key] = _build(dt, n_blocks, kit, jl)
    return _cache[key]


LAST_RESULTS = None  # BassKernelResults of the most recent run (for profiling)


def kernel(u, dt, W, M, H, b, _trace=False):
    from concourse.bass_utils import run_bass_kernel_spmd

    dt_f = float(np.asarray(dt).reshape(-1)[0])
    nc = _get_nc(dt_f, NBLK, K, JL)

    W = np.asarray(W, np.float32)
    M = np.asarray(M, np.float32)
    H = np.asarray(H, np.float32)

    def tiles16(A, scale):
        # [P, 2, C] with [p, kh, j] = (scale*A)[j, kh*P + p]
        AT = (scale * A).T.reshape(2, P, C).transpose(1, 0, 2)
        return np.ascontiguousarray(AT).astype(np.float16)

    u32 = np.asarray(u, np.float32).reshape(INQ, 2, P, BPI, B)
    u16 = np.ascontiguousarray(u32.transpose(0, 3, 2, 1, 4).reshape(NBLK, P, 2, B))
    in_map = {
        "u16": u16.astype(np.float16),
        "wt16": tiles16(W, dt_f),
        "mt16": tiles16(M, dt_f),
        "ht16": tiles16(H, 1.0),
        "ident": np.eye(P, dtype=np.float16),
        "bvec": np.ascontiguousarray(np.asarray(b, np.float32).reshape(2, P).T),
    }
    res = run_bass_kernel_spmd(nc, [in_map], core_ids=[0], trace=_trace)
    global LAST_RESULTS
    LAST_RESULTS = res
    out = res.results[0]

    def untile(arr):
        a5 = arr.reshape(INQ, BPI, P, 2, B).transpose(0, 3, 2, 1, 4)
        return np.ascontiguousarray(a5.reshape(INQ, C, T)).astype(np.float32)

    return untile(out["ys16"]), untile(out["xs16"])


# revision 8
# speedup vs baseline: 1.1915x; 1.1915x over previous
"""Trainium2 Bass kernel for the leaky-tanh recurrent network (CombinedModel).

Math (reference):
    u_seq = u.transpose(0,2,1).reshape(Iq*T, C)      # state persists across inquiries
    x_t = (1-dt)*x_{t-1} + (dt*W) @ tanh(x_{t-1}) + (dt*M) @ u_t
    y_t = sigmoid(H @ x_t + b)

Strategy: blocks of B=256 steps solved by Picard iteration: given a guess
trajectory X, the refinement is the affine scan x_t = a*x_{t-1} + (G_t+V_t)
with G = (dt*W) @ tanh(X_shifted), V = (dt*M) @ U.  The scan is one
DVE/Pool tensor_tensor_scan per 128-channel half; G and V are PE matmuls
with all inputs in fp16 (1 PE cycle/row vs 4 for fp32; scans, state and
PSUM accumulation stay fp32 -- validated to ~2e-3 max err vs the 2e-2
tolerance in a bit-accurate numpy mimic).

The cross-block serial chain is the block anchor handoff.  The dynamics
sit at the edge of chaos (anchor errors are amplified, never damped), so
anchors must come from deeply converged states: block b+1 starts from
block b's state after iteration K-JL (the "stale" anchor) and runs its
first K-JL iterations on it; the last JL iterations re-anchor on block
b's final state, which rides a second, shallow serial chain.  K=11/JL=2
measures ~2e-3 max error in the mimic.

Per-iteration chain latency is minimized by keeping each concurrent
stage's operands in separate tiles -- the tile framework serializes
readers of a shared recycled pool buffer, so each channel half gets its
own PSUM pool, x tile and tanh tile; the h0 scan runs on DVE while the
h1 scan runs on Pool; tanh is split per half (h0 first, since its scan
finishes first and its result gates the kh0 matmuls that hide under
tanh-h1); the kh1 matmul pair closes pc0 before pc1.
"""

import numpy as np

INQ, C, T = 64, 256, 1024
B = 256            # picard block length
P = 128            # partitions; C = 2*P channel halves
K = 11             # picard refinement iterations per block
JL = 2             # late iterations re-anchored on the exact initial state
NBLK = INQ * T // B
BPI = T // B       # blocks per inquiry

_cache: dict = {}


def _build(dt: float, n_blocks: int, kit: int, jl: int):
    import concourse.bacc as bacc
    import concourse.mybir as mybir
    from concourse.tile import TileContext

    f32 = mybir.dt.float32
    f16 = mybir.dt.float16
    AF = mybir.ActivationFunctionType
    ALU = mybir.AluOpType
    a = 1.0 - dt

    nc = bacc.Bacc(None, target_bir_lowering=False, debug=False)
    u_in = nc.dram_tensor("u16", [NBLK, P, 2, B], f16, kind="ExternalInput")
    wt_in = nc.dram_tensor("wt16", [P, 2, C], f16, kind="ExternalInput")
    mt_in = nc.dram_tensor("mt16", [P, 2, C], f16, kind="ExternalInput")
    ht_in = nc.dram_tensor("ht16", [P, 2, C], f16, kind="ExternalInput")
    id_in = nc.dram_tensor("ident", [P, P], f16, kind="ExternalInput")
    b_in = nc.dram_tensor("bvec", [P, 2], f32, kind="ExternalInput")
    ys_out = nc.dram_tensor("ys16", [NBLK, P, 2, B], f16, kind="ExternalOutput")
    xs_out = nc.dram_tensor("xs16", [NBLK, P, 2, B], f16, kind="ExternalOutput")

    with TileContext(nc) as tc:
        with (
            tc.tile_pool(name="const", bufs=1) as cpool,
            tc.tile_pool(name="io", bufs=6) as iopool,
            tc.tile_pool(name="xp0", bufs=4) as xpool0,
            tc.tile_pool(name="xp1", bufs=4) as xpool1,
            tc.tile_pool(name="txp0", bufs=3) as txpool0,
            tc.tile_pool(name="txp1", bufs=3) as txpool1,
            tc.tile_pool(name="vp0", bufs=3) as vpool0,
            tc.tile_pool(name="vp1", bufs=3) as vpool1,
            tc.tile_pool(name="x16p", bufs=2) as x16pool,
            tc.tile_pool(name="yp", bufs=2) as ypool,
            tc.tile_pool(name="pc0", bufs=2, space="PSUM") as pc0pool,
            tc.tile_pool(name="pc1", bufs=2, space="PSUM") as pc1pool,
            tc.tile_pool(name="pv0", bufs=1, space="PSUM") as pv0pool,
            tc.tile_pool(name="pv1", bufs=1, space="PSUM") as pv1pool,
            tc.tile_pool(name="py", bufs=2, space="PSUM") as pypool,
        ):
            w_sb = cpool.tile([P, 2, C], f16, tag="w")
            m_sb = cpool.tile([P, 2, C], f16, tag="m")
            h_sb = cpool.tile([P, 2, C], f16, tag="h")
            i_sb = cpool.tile([P, P], f16, tag="i")
            b_sb = cpool.tile([P, 2], f32, tag="b")
            a_sb = cpool.tile([P, B], f32, tag="a")
            nc.sync.dma_start(w_sb[:, :, :], wt_in[:, :, :])
            nc.sync.dma_start(m_sb[:, :, :], mt_in[:, :, :])
            nc.sync.dma_start(h_sb[:, :, :], ht_in[:, :, :])
            nc.sync.dma_start(i_sb[:, :], id_in[:, :])
            nc.sync.dma_start(b_sb[:, :], b_in[:, :])
            nc.vector.memset(a_sb[:, :], a)

            def emit_load(blk):
                u_t = iopool.tile([P, 2, B], f16, tag="u")
                nc.sync.dma_start(u_t[:, :, :], u_in[blk])
                return u_t

            def emit_drive(u_t):
                # V = (dt*M) @ U into psum; fp16 SBUF copies for I@V injection
                pv0 = pv0pool.tile([P, B], f32, tag="pv0")
                pv1 = pv1pool.tile([P, B], f32, tag="pv1")
                pvs = (pv0, pv1)
                for mh in range(2):
                    for kh in range(2):
                        nc.tensor.matmul(
                            pvs[mh][:, :],
                            m_sb[:, kh, mh * P : (mh + 1) * P],
                            u_t[:, kh, :],
                            start=(kh == 0),
                            stop=(kh == 1),
                        )
                v0 = vpool0.tile([P, B], f16, tag="v0")
                v1 = vpool1.tile([P, B], f16, tag="v1")
                nc.gpsimd.tensor_scalar_add(v0[:, :], pv0[:, :], 0.0)
                nc.gpsimd.tensor_scalar_add(v1[:, :], pv1[:, :], 0.0)
                return pv0, pv1, v0, v1

            def emit_anchor(x0, x1, xp):
                # col0 <- anchor: end column of the previous block's halves
                # (h0 via DVE, h1 via Pool -- same engines as their writers),
                # or zero for block 0.
                if xp is None:
                    nc.vector.memset(x0[:, 0:1], 0.0)
                    nc.gpsimd.memset(x1[:, 0:1], 0.0)
                else:
                    nc.vector.tensor_scalar_add(x0[:, 0:1], xp[0][:, B : B + 1], 0.0)
                    nc.gpsimd.tensor_scalar_add(x1[:, 0:1], xp[1][:, B : B + 1], 0.0)

            def emit_scan0(pv0, pv1, xp):
                x0 = xpool0.tile([P, B + 1], f32, tag="x0")
                x1 = xpool1.tile([P, B + 1], f32, tag="x1")
                emit_anchor(x0, x1, xp)
                nc.vector.tensor_tensor_scan(
                    x0[:, 1 : B + 1], a_sb[:, :], pv0[:, :],
                    x0[:, 0:1], ALU.mult, ALU.add,
                )
                nc.gpsimd.tensor_tensor_scan(
                    x1[:, 1 : B + 1], a_sb[:, :], pv1[:, :],
                    x1[:, 0:1], ALU.mult, ALU.add,
                )
                return x0, x1

            def emit_iter(x0, x1, v0, v1):
                pc0 = pc0pool.tile([P, B], f32, tag="pc0")
                pc1 = pc1pool.tile([P, B], f32, tag="pc1")
                nc.tensor.matmul(pc0[:, :], i_sb[:, :], v0[:, :], start=True, stop=False)
                nc.tensor.matmul(pc1[:, :], i_sb[:, :], v1[:, :], start=True, stop=False)
                tx0 = txpool0.tile([P, B], f16, tag="tx0")
                tx1 = txpool1.tile([P, B], f16, tag="tx1")
                nc.scalar.activation(tx0[:, :], x0[:, 0:B], AF.Tanh)
                nc.scalar.activation(tx1[:, :], x1[:, 0:B], AF.Tanh)
                txs = (tx0, tx1)
                pcs = (pc0, pc1)
                # kh0 matmuls hide under tanh-h1; the kh1 pair closes pc0
                # first so the h0 scan (DVE, gating the next tanh) starts
                # as early as possible.
                for kh in range(2):
                    for mh in range(2):
                        nc.tensor.matmul(
                            pcs[mh][:, :],
                            w_sb[:, kh, mh * P : (mh + 1) * P],
                            txs[kh][:, :],
                            start=False,
                            stop=(kh == 1),
                        )
                nc.vector.tensor_tensor_scan(
                    x0[:, 1 : B + 1], a_sb[:, :], pc0[:, :],
                    x0[:, 0:1], ALU.mult, ALU.add,
                )
                nc.gpsimd.tensor_tensor_scan(
                    x1[:, 1 : B + 1], a_sb[:, :], pc1[:, :],
                    x1[:, 0:1], ALU.mult, ALU.add,
                )

            def emit_readout(blk, x0, x1):
                x16 = x16pool.tile([P, 2, B], f16, tag="x16")
                nc.vector.tensor_scalar_add(x16[:, 0, :], x0[:, 1 : B + 1], 0.0)
                nc.vector.tensor_scalar_add(x16[:, 1, :], x1[:, 1 : B + 1], 0.0)
                py = pypool.tile([P, 2 * B], f32, tag="py")
                for mh in range(2):
                    for kh in range(2):
                        nc.tensor.matmul(
                            py[:, mh * B : (mh + 1) * B],
                            h_sb[:, kh, mh * P : (mh + 1) * P],
                            x16[:, kh, :],
                            start=(kh == 0),
                            stop=(kh == 1),
                        )
                y16 = ypool.tile([P, 2, B], f16, tag="y")
                for mh in range(2):
                    nc.scalar.activation(
                        y16[:, mh, :], py[:, mh * B : (mh + 1) * B],
                        AF.Sigmoid, bias=b_sb[:, mh : mh + 1],
                    )
                nc.sync.dma_start(ys_out[blk], y16[:, :, :])
                nc.sync.dma_start(xs_out[blk], x16[:, :, :])

            # ---- software-pipelined emission over blocks ---------------
            # round b: stale iterations of b (the serial chain), then hand
            # the chain to b+1 (anchor copy + drive + scan0), then the
            # late phase + readout of b-1 as filler that executes inside
            # the chain's dependency stalls.
            u_hand = {}
            for blk in range(min(4, n_blocks)):
                u_hand[blk] = emit_load(blk)
            d = emit_drive(u_hand[0])
            xs_of = {0: emit_scan0(d[0], d[1], None)}
            vs_of = {0: (d[2], d[3])}
            for b in range(n_blocks):
                x0, x1 = xs_of[b]
                v0, v1 = vs_of[b]
                for k in range(kit - jl):
                    emit_iter(x0, x1, v0, v1)
                if b + 1 < n_blocks:
                    if b + 4 < n_blocks:
                        u_hand[b + 4] = emit_load(b + 4)
                    dn = emit_drive(u_hand[b + 1])
                    vs_of[b + 1] = (dn[2], dn[3])
                    xs_of[b + 1] = emit_scan0(dn[0], dn[1], (x0, x1))
                if b > 0:
                    xp0, xp1 = xs_of[b - 1]
                    emit_anchor(xp0, xp1, xs_of.get(b - 2))
                    for k in range(jl):
                        emit_iter(xp0, xp1, *vs_of[b - 1])
                    emit_readout(b - 1, xp0, xp1)
                    xs_of.pop(b - 2, None)
                    vs_of.pop(b - 1, None)
                    u_hand.pop(b - 1, None)
            bl = n_blocks - 1
            xp0, xp1 = xs_of[bl]
            emit_anchor(xp0, xp1, xs_of.get(bl - 1))
            for k in range(jl):
                emit_iter(xp0, xp1, *vs_of[bl])
            emit_readout(bl, xp0, xp1)

    nc.compile()
    return nc


def _get_nc(dt: float, n_blocks: int, kit: int, jl: int):
    key = (dt, n_blocks, kit, jl)
    if key not in _cache:
        _cache[key] = _build(dt, n_blocks, kit, jl)
    return _cache[key]


LAST_RESULTS = None  # BassKernelResults of the most recent run (for profiling)


def kernel(u, dt, W, M, H, b, _trace=False):
    from concourse.bass_utils import run_bass_kernel_spmd

    dt_f = float(np.asarray(dt).reshape(-1)[0])
    nc = _get_nc(dt_f, NBLK, K, JL)

    W = np.asarray(W, np.float32)
    M = np.asarray(M, np.float32)
    H = np.asarray(H, np.float32)

    def tiles16(A, scale):
        # [P, 2, C] with [p, kh, j] = (scale*A)[j, kh*P + p]
        AT = (scale * A).T.reshape(2, P, C).transpose(1, 0, 2)
        return np.ascontiguousarray(AT).astype(np.float16)

    u32 = np.asarray(u, np.float32).reshape(INQ, 2, P, BPI, B)
    u16 = np.ascontiguousarray(u32.transpose(0, 3, 2, 1, 4).reshape(NBLK, P, 2, B))
    in_map = {
        "u16": u16.astype(np.float16),
        "wt16": tiles16(W, dt_f),
        "mt16": tiles16(M, dt_f),
        "ht16": tiles16(H, 1.0),
        "ident": np.eye(P, dtype=np.float16),
        "bvec": np.ascontiguousarray(np.asarray(b, np.float32).reshape(2, P).T),
    }
    res = run_bass_kernel_spmd(nc, [in_map], core_ids=[0], trace=_trace)
    global LAST_RESULTS
    LAST_RESULTS = res
    out = res.results[0]

    def untile(arr):
        a5 = arr.reshape(INQ, BPI, P, 2, B).transpose(0, 3, 2, 1, 4)
        return np.ascontiguousarray(a5.reshape(INQ, C, T)).astype(np.float32)

    return untile(out["ys16"]), untile(out["xs16"])
